# revision 60
# baseline (speedup 1.0000x reference)
"""Trainium2 Bass kernel for the Antecedent (fuzzy firing strength) problem.

fir[s, r] = exp(sum_d logmv[s, fs_ind[r, d], d])
with logmv[s, f, d] = -(x[s,d] - c[f,d])^2 / (2 * spread[f,d]^2)

The gather+sum over d is a matmul with contraction K = num_fs*in_dim = 32:
    fir[s, r] = exp( sum_k oh[k, r] * d2sq[k, s] ),
    oh[f*8+d, r]   = -1 iff fs_ind[r, d] == f, else 0 (host-built bf16 index
                     encoding; the -1 carries the gaussian exponent's sign)
    d2sq[f*8+d, s] = ((x[s,d]-c[f,d]) / (spread[f,d]*sqrt(2)))^2
                     (device-computed from x/center/spread)

Sharding: rules split across the 8 cores (8192 rules each); samples replicated.
Per core: 64 bf16 matmuls [K=32, M=128 samples, N=512 rules] -> f32 PSUM,
ScalarE Exp PSUM[128,2048] -> bf16 SBUF, 0.5MB DMAs to the [512, 8192] output
slice (bf16, upcast to f32 on the host). Steady state is ScalarE-bound: exp
runs at 1 elem/cycle/lane, 4.2M output elems/core ~= 31us, with matmul (~90%)
and output DMA (~60%) hidden under it.
"""

import sys

if "/opt/trn_rl_repo" not in sys.path:
    sys.path.insert(0, "/opt/trn_rl_repo")

import ml_dtypes
import numpy as np

import concourse.bacc as bacc
import concourse.mybir as mybir
import concourse.tile as tile
from concourse.bass_utils import run_bass_kernel_spmd
from concourse.tile_rust import add_dep_helper

NUM_SAM = 512
IN_DIM = 8
NUM_FS = 4
NUM_RULE = 65536
K = NUM_FS * IN_DIM  # 32 contraction size
N_CORES = 8
RPC = NUM_RULE // N_CORES  # 8192 rules per core

F32 = mybir.dt.float32
BF16 = mybir.dt.bfloat16
OUT_DT = BF16  # fir values are exp(<=0) in (0,1]; bf16 keeps rel err ~1e-3
DEBUG_SKIP_AB_EXP = False

# loop tiling (per core)
N_SG = NUM_SAM // 128          # 4 sample groups of 128 (partition dim)
N_MM = 4                       # matmuls per exp group (512 rules)
MM_N = 512                     # moving free dim per matmul
EXP_N = N_MM * MM_N            # 2048 rules per exp + output DMA group
N_GRP = RPC // EXP_N           # 4 groups per sample group


def build_nc(fact):
    nc = bacc.Bacc("TRN2", target_bir_lowering=False, debug=False, num_devices=N_CORES)

    oh_ext = nc.dram_tensor("onehot", [K, RPC], BF16, kind="ExternalInput")
    # xcs: cols 0..NUM_SAM-1 = x[s,d] repeated over f; col NUM_SAM = center,
    # col NUM_SAM+1 = spread (single input DMA for the whole prologue)
    xcs_ext = nc.dram_tensor("xcs", [K, NUM_SAM + 2], F32, kind="ExternalInput")
    if fact:
        oha_ext = nc.dram_tensor("oha", [K // 2, HI_PC], BF16, kind="ExternalInput")
        ohb_ext = nc.dram_tensor("ohb", [K // 2, N_LO], BF16, kind="ExternalInput")
    out_ext = nc.dram_tensor("out", [NUM_SAM, RPC], OUT_DT, kind="ExternalOutput")

    with tile.TileContext(nc) as tc:
        with (
            tc.tile_pool(name="const", bufs=1) as cpool,
            tc.tile_pool(name="stage", bufs=4) as spool,
            tc.tile_pool(name="psum", bufs=2, space="PSUM") as ppool,
        ):
            # ---- prologue: tiny inputs + membership table ----
            xcs = cpool.tile([K, NUM_SAM + 2], F32)
            nc.sync.dma_start(out=xcs[:], in_=xcs_ext[:])
            xt32 = xcs[:, 0:NUM_SAM]
            cvec = xcs[:, NUM_SAM : NUM_SAM + 1]
            svec = xcs[:, NUM_SAM + 1 : NUM_SAM + 2]

            if fact:
                # factor one-hots, plus a base-partition-0 copy of the
                # factor-B rows of xcs (matmul operands need bp 0)
                oha = cpool.tile([K // 2, HI_PC], BF16)
                ohb = cpool.tile([K // 2, N_LO], BF16)
                xcs2 = cpool.tile([K // 2, NUM_SAM + 2], F32)
                nc.sync.dma_start(out=xcs2[:], in_=xcs_ext[K // 2 : K, :])
                nc.sync.dma_start(out=oha[:], in_=oha_ext[:])
                nc.sync.dma_start(out=ohb[:], in_=ohb_ext[:])

            # one-hot rule encoding [K, RPC] with entries -1 (carries the
            # minus sign of the gaussian exponent); issued on the Scalar
            # HWDGE queue so it doesn't serialize behind Sync's const DMA
            oh = cpool.tile([K, RPC], BF16)
            c0 = 0
            for csz in (MM_N, 2560, 2560, 2560):  # small first chunk: the
                nc.scalar.dma_start(             # first matmul starts sooner
                    out=oh[:, c0 : c0 + csz],
                    in_=oh_ext[:, c0 : c0 + csz],
                )
                c0 += csz

            # d2[k, s] = (x - c) / (s * sqrt(2)); lhsT = d2^2 (bf16).
            # The exponent's minus sign lives in the -1 one-hot entries.
            rsv = cpool.tile([K, 1], F32)
            tvec = cpool.tile([K, 1], F32)
            nc.vector.reciprocal(rsv[:], svec)
            nc.vector.tensor_scalar_mul(tvec[:], rsv[:], 0.7071067811865476)
            d2 = cpool.tile([K, NUM_SAM], F32)
            lhs_b = cpool.tile([K, NUM_SAM], BF16)
            # sample-group 0 first so its matmuls can start early
            for sl in (slice(0, 128), slice(128, NUM_SAM)):
                nc.vector.tensor_scalar(
                    d2[:, sl], xt32[:, sl], cvec, tvec[:],
                    mybir.AluOpType.subtract, mybir.AluOpType.mult,
                )
                nc.vector.tensor_mul(lhs_b[:, sl], d2[:, sl], d2[:, sl])

            Exp = mybir.ActivationFunctionType.Exp

            lhsB = None
            if fact:
                rsv2 = cpool.tile([K // 2, 1], F32)
                tvec2 = cpool.tile([K // 2, 1], F32)
                nc.vector.reciprocal(rsv2[:], xcs2[:, NUM_SAM + 1 : NUM_SAM + 2])
                nc.vector.tensor_scalar_mul(tvec2[:], rsv2[:], 0.7071067811865476)
                d2b = cpool.tile([K // 2, NUM_SAM], F32)
                lhsB = cpool.tile([K // 2, NUM_SAM], BF16)
                for sl in (slice(0, 128), slice(128, NUM_SAM)):
                    nc.vector.tensor_scalar(
                        d2b[:, sl], xcs2[:, sl],
                        xcs2[:, NUM_SAM : NUM_SAM + 1], tvec2[:],
                        mybir.AluOpType.subtract, mybir.AluOpType.mult,
                    )
                    nc.vector.tensor_mul(lhsB[:, sl], d2b[:, sl], d2b[:, sl])

            ab_tiles = []
            last_ab_mm = last_ab_exp = None
            if fact:
                # A/B tables: per sg, A sums at psum cols [sg*512, +32),
                # B sums at [sg*512+32, +288) (one bank per sg), then one
                # exp per sg -> ab_sb[sg] = [A (32 cols) | B (256 cols)] bf16
                ps_ab = ppool.tile([128, EXP_N], F32, tag="ps")
                for sg in range(N_SG):
                    s0 = sg * 512
                    nc.tensor.matmul(
                        ps_ab[:, s0 : s0 + HI_PC],
                        lhs_b[0 : K // 2, sg * 128 : (sg + 1) * 128],
                        oha[:],
                        start=True,
                        stop=True,
                    )
                    last_ab_mm = nc.tensor.matmul(
                        ps_ab[:, s0 + HI_PC : s0 + HI_PC + N_LO],
                        lhsB[:, sg * 128 : (sg + 1) * 128],
                        ohb[:],
                        start=True,
                        stop=True,
                    )
                for sg in range(N_SG):
                    ab = cpool.tile([128, HI_PC + N_LO], BF16, tag=f"ab{sg}")
                    s0 = sg * 512
                    if DEBUG_SKIP_AB_EXP:
                        nc.vector.memset(ab[:], 0.5)
                    else:
                        last_ab_exp = nc.scalar.activation(
                            ab[:], ps_ab[:, s0 : s0 + HI_PC + N_LO], Exp
                        )
                    ab_tiles.append(ab)

            # ---- main loop ----
            # fact: groups 0,2 via onehot-matmul + ACT exp; groups 1,3 via
            # DVE broadcast multiply A[s,hi]*B[s,lo] (no exp, no big matmul)
            dve_groups = {1, 3} if fact else set()
            for sg in range(N_SG):
                lhsT = lhs_b[0:K, sg * 128 : (sg + 1) * 128]  # [32, 128]
                for g in range(N_GRP):
                    stg = spool.tile([128, EXP_N], OUT_DT)
                    out_slice = out_ext[
                        sg * 128 : (sg + 1) * 128, g * EXP_N : (g + 1) * EXP_N
                    ]
                    if g in dve_groups:
                        ab = ab_tiles[sg]
                        Ab = (
                            ab[:, g * 8 : (g + 1) * 8]
                            .rearrange("p (h o) -> p h o", o=1)
                            .broadcast_to([128, 8, N_LO])
                        )
                        Bb = (
                            ab[:, HI_PC : HI_PC + N_LO]
                            .rearrange("p (o n) -> p o n", o=1)
                            .broadcast_to([128, 8, N_LO])
                        )
                        o3 = stg[:].rearrange("p (h n) -> p h n", h=8)
                        nc.vector.tensor_tensor(o3, Bb, Ab, mybir.AluOpType.mult)
                        if sg == N_SG - 1 and g == N_GRP - 1:
                            hlf = EXP_N // 2
                            nc.sync.dma_start(
                                out=out_slice[:, :hlf], in_=stg[:, :hlf]
                            )
                            nc.sync.dma_start(
                                out=out_slice[:, hlf:], in_=stg[:, hlf:]
                            )
                        else:
                            nc.sync.dma_start(out=out_slice, in_=stg[:])
                        continue
                    ps = ppool.tile([128, EXP_N], F32, tag="ps")
                    for j in range(N_MM):
                        rt = g * N_MM + j
                        mm = nc.tensor.matmul(
                            ps[:, j * MM_N : (j + 1) * MM_N],
                            lhsT,
                            oh[:, rt * MM_N : (rt + 1) * MM_N],
                            start=True,
                            stop=True,
                        )
                        if last_ab_mm is not None:
                            # keep the AB phase ahead of the loop on the PE
                            # stream so the shared psum slot can't deadlock
                            add_dep_helper(
                                mm.ins, last_ab_mm.ins, sync=False,
                                reason="AB tables before rule matmuls",
                            )
                            last_ab_mm = None
                    if last_ab_exp is not None:
                        ex = nc.scalar.activation(
                            stg[:, 0:MM_N], ps[:, 0:MM_N], Exp
                        )
                        add_dep_helper(
                            ex.ins, last_ab_exp.ins, sync=False,
                            reason="AB exps before rule exps",
                        )
                        last_ab_exp = None
                        nc.scalar.activation(stg[:, MM_N:], ps[:, MM_N:], Exp)
                        nc.sync.dma_start(out=out_slice, in_=stg[:])
                    elif sg == N_SG - 1 and g == N_GRP - 1:
                        nc.scalar.activation(stg[:], ps[:], Exp)
                        # two half DMAs run concurrently -> shorter drain tail
                        h = EXP_N // 2
                        nc.sync.dma_start(out=out_slice[:, :h], in_=stg[:, :h])
                        nc.sync.dma_start(out=out_slice[:, h:], in_=stg[:, h:])
                    else:
                        nc.scalar.activation(stg[:], ps[:], Exp)
                        nc.sync.dma_start(out=out_slice, in_=stg[:])

    nc.compile()
    return nc


KX = K + K // 2       # xcs rows: 32 standard + 16 duplicated factor-B rows
D_A = IN_DIM // 2     # factor A: dims 0..3 (k rows 0..15)
N_HI = NUM_FS**D_A    # 256 A-codes; per core 32 hi blocks
N_LO = NUM_FS**D_A    # 256 B-codes
HI_PC = RPC // N_LO   # 32 hi blocks per core


def _is_factorizable(fs):
    """fs[r, 0:4] depends only on r>>8 and fs[r, 4:8] only on r&255
    (true for the FuCo-FRB cartesian rule base)."""
    a = fs[:, :D_A].reshape(N_HI, N_LO, D_A)
    b = fs[:, D_A:].reshape(N_HI, N_LO, D_A)
    return bool((a == a[:, :1]).all() and (b == b[:1]).all())


def _prep_in_maps(model_input, center, spread, fs_ind):
    model_input = np.ascontiguousarray(model_input, dtype=np.float32)
    center = np.ascontiguousarray(center, dtype=np.float32)
    spread = np.ascontiguousarray(spread, dtype=np.float32)
    fs = np.clip(np.asarray(fs_ind), 0, NUM_FS - 1).astype(np.int64)

    # one-hot with k = d*NUM_FS + f rows: oh[k, r] = -1 iff fs_ind[r, d] == f
    # (the -1 carries the gaussian exponent's sign)
    oh = np.zeros((K, NUM_RULE), dtype=ml_dtypes.bfloat16)
    r = np.arange(NUM_RULE)
    for d in range(IN_DIM):
        oh[d * NUM_FS + fs[:, d], r] = -1.0

    # xcs: x transposed/repeated over f, plus center and spread columns
    # (row k = d*4+f holds x[s, d], center[f, d], spread[f, d]).
    # Rows 32..47 duplicate rows 16..31 (factor-B dims) so the fast path's
    # K=16 B-matmul can run at base partition 32 (tile_position constraint).
    xcs = np.empty((K, NUM_SAM + 2), dtype=np.float32)
    xcs[:, :NUM_SAM] = np.repeat(model_input.T, NUM_FS, axis=0)
    xcs[:, NUM_SAM] = center.T.reshape(K)
    xcs[:, NUM_SAM + 1] = spread.T.reshape(K)

    fact = _is_factorizable(fs)
    oha = ohb = None
    if fact:
        # A-table one-hot [16, 256 hi codes], B-table one-hot [16, 256]
        oha = np.zeros((K // 2, N_HI), dtype=ml_dtypes.bfloat16)
        ohb = np.zeros((K // 2, N_LO), dtype=ml_dtypes.bfloat16)
        hi_rep = fs[:: N_LO, :D_A]  # [256, 4] representative rows
        lo_rep = fs[:N_LO, D_A:]    # [256, 4]
        for d in range(D_A):
            oha[d * NUM_FS + hi_rep[:, d], np.arange(N_HI)] = -1.0
            ohb[d * NUM_FS + lo_rep[:, d], np.arange(N_LO)] = -1.0

    maps = []
    for i in range(N_CORES):
        m = {
            "onehot": np.ascontiguousarray(oh[:, i * RPC : (i + 1) * RPC]),
            "xcs": xcs,
        }
        if fact:
            m["oha"] = np.ascontiguousarray(oha[:, i * HI_PC : (i + 1) * HI_PC])
            m["ohb"] = ohb
        maps.append(m)
    return fact, maps


def _run(inputs, trace=False, **spmd_kwargs):
    fact, in_maps = _prep_in_maps(
        inputs["model_input"], inputs["center"], inputs["spread"], inputs["fs_ind"]
    )
    nc = build_nc(fact)
    res = run_bass_kernel_spmd(
        nc, in_maps, core_ids=list(range(N_CORES)), trace=trace, **spmd_kwargs
    )
    out = np.concatenate(
        [res.results[i]["out"].astype(np.float32) for i in range(N_CORES)], axis=1
    )
    return out, res


def kernel(model_input, center, spread, fs_ind):
    out, _ = _run(
        {
            "model_input": model_input,
            "center": center,
            "spread": spread,
            "fs_ind": fs_ind,
        }
    )
    return out


# revision 61
# speedup vs baseline: 1.0023x; 1.0023x over previous
"""Trainium2 Bass kernel for the Antecedent (fuzzy firing strength) problem.

fir[s, r] = exp(sum_d logmv[s, fs_ind[r, d], d])
with logmv[s, f, d] = -(x[s,d] - c[f,d])^2 / (2 * spread[f,d]^2)

The gather+sum over d is a matmul with contraction K = num_fs*in_dim = 32:
    fir[s, r] = exp( sum_k oh[k, r] * d2sq[k, s] ),
    oh[f*8+d, r]   = -1 iff fs_ind[r, d] == f, else 0 (host-built bf16 index
                     encoding; the -1 carries the gaussian exponent's sign)
    d2sq[f*8+d, s] = ((x[s,d]-c[f,d]) / (spread[f,d]*sqrt(2)))^2
                     (device-computed from x/center/spread)

Sharding: rules split across the 8 cores (8192 rules each); samples replicated.
Per core: 64 bf16 matmuls [K=32, M=128 samples, N=512 rules] -> f32 PSUM,
ScalarE Exp PSUM[128,2048] -> bf16 SBUF, 0.5MB DMAs to the [512, 8192] output
slice (bf16, upcast to f32 on the host). Steady state is ScalarE-bound: exp
runs at 1 elem/cycle/lane, 4.2M output elems/core ~= 31us, with matmul (~90%)
and output DMA (~60%) hidden under it.
"""

import sys

if "/opt/trn_rl_repo" not in sys.path:
    sys.path.insert(0, "/opt/trn_rl_repo")

import ml_dtypes
import numpy as np

import concourse.bacc as bacc
import concourse.mybir as mybir
import concourse.tile as tile
from concourse.bass_utils import run_bass_kernel_spmd
from concourse.tile_rust import add_dep_helper

NUM_SAM = 512
IN_DIM = 8
NUM_FS = 4
NUM_RULE = 65536
K = NUM_FS * IN_DIM  # 32 contraction size
N_CORES = 8
RPC = NUM_RULE // N_CORES  # 8192 rules per core

F32 = mybir.dt.float32
BF16 = mybir.dt.bfloat16
OUT_DT = BF16  # fir values are exp(<=0) in (0,1]; bf16 keeps rel err ~1e-3
DEBUG_SKIP_AB_EXP = False

# loop tiling (per core)
N_SG = NUM_SAM // 128          # 4 sample groups of 128 (partition dim)
N_MM = 4                       # matmuls per exp group (512 rules)
MM_N = 512                     # moving free dim per matmul
EXP_N = N_MM * MM_N            # 2048 rules per exp + output DMA group
N_GRP = RPC // EXP_N           # 4 groups per sample group


def build_nc(fact):
    nc = bacc.Bacc("TRN2", target_bir_lowering=False, debug=False, num_devices=N_CORES)

    oh_ext = nc.dram_tensor("onehot", [K, RPC], BF16, kind="ExternalInput")
    # xcs: cols 0..NUM_SAM-1 = x[s,d] repeated over f; col NUM_SAM = center,
    # col NUM_SAM+1 = spread (single input DMA for the whole prologue)
    xcs_ext = nc.dram_tensor("xcs", [K, NUM_SAM + 2], F32, kind="ExternalInput")
    if fact:
        oha_ext = nc.dram_tensor("oha", [K // 2, HI_PC], BF16, kind="ExternalInput")
        ohb_ext = nc.dram_tensor("ohb", [K // 2, N_LO], BF16, kind="ExternalInput")
    out_ext = nc.dram_tensor("out", [NUM_SAM, RPC], OUT_DT, kind="ExternalOutput")

    with tile.TileContext(nc) as tc:
        with (
            tc.tile_pool(name="const", bufs=1) as cpool,
            tc.tile_pool(name="stage", bufs=4) as spool,
            tc.tile_pool(name="psum", bufs=2, space="PSUM") as ppool,
        ):
            # ---- prologue: tiny inputs + membership table ----
            xcs = cpool.tile([K, NUM_SAM + 2], F32)
            nc.sync.dma_start(out=xcs[:], in_=xcs_ext[:])
            xt32 = xcs[:, 0:NUM_SAM]
            cvec = xcs[:, NUM_SAM : NUM_SAM + 1]
            svec = xcs[:, NUM_SAM + 1 : NUM_SAM + 2]

            if fact:
                # factor one-hots, plus a base-partition-0 copy of the
                # factor-B rows of xcs (matmul operands need bp 0)
                oha = cpool.tile([K // 2, HI_PC], BF16)
                ohb = cpool.tile([K // 2, N_LO], BF16)
                xcs2 = cpool.tile([K // 2, NUM_SAM + 2], F32)
                nc.sync.dma_start(out=oha[:], in_=oha_ext[:])
                nc.sync.dma_start(out=ohb[:], in_=ohb_ext[:])
                nc.sync.dma_start(out=xcs2[:], in_=xcs_ext[K // 2 : K, :])

            # one-hot rule encoding [K, RPC] with entries -1 (carries the
            # minus sign of the gaussian exponent); issued on the Scalar
            # HWDGE queue so it doesn't serialize behind Sync's const DMA
            oh = cpool.tile([K, RPC], BF16)
            c0 = 0
            for csz in (MM_N, 2560, 2560, 2560):  # small first chunk: the
                nc.scalar.dma_start(             # first matmul starts sooner
                    out=oh[:, c0 : c0 + csz],
                    in_=oh_ext[:, c0 : c0 + csz],
                )
                c0 += csz

            # d2[k, s] = (x - c) / (s * sqrt(2)); lhsT = d2^2 (bf16).
            # The exponent's minus sign lives in the -1 one-hot entries.
            rsv = cpool.tile([K, 1], F32)
            tvec = cpool.tile([K, 1], F32)
            nc.vector.reciprocal(rsv[:], svec)
            nc.vector.tensor_scalar_mul(tvec[:], rsv[:], 0.7071067811865476)
            d2 = cpool.tile([K, NUM_SAM], F32)
            lhs_b = cpool.tile([K, NUM_SAM], BF16)
            # sample-group 0 first so its matmuls can start early
            for sl in (slice(0, 128), slice(128, NUM_SAM)):
                nc.vector.tensor_scalar(
                    d2[:, sl], xt32[:, sl], cvec, tvec[:],
                    mybir.AluOpType.subtract, mybir.AluOpType.mult,
                )
                nc.vector.tensor_mul(lhs_b[:, sl], d2[:, sl], d2[:, sl])

            Exp = mybir.ActivationFunctionType.Exp

            lhsB = None
            if fact:
                rsv2 = cpool.tile([K // 2, 1], F32)
                tvec2 = cpool.tile([K // 2, 1], F32)
                nc.vector.reciprocal(rsv2[:], xcs2[:, NUM_SAM + 1 : NUM_SAM + 2])
                nc.vector.tensor_scalar_mul(tvec2[:], rsv2[:], 0.7071067811865476)
                d2b = cpool.tile([K // 2, NUM_SAM], F32)
                lhsB = cpool.tile([K // 2, NUM_SAM], BF16)
                nc.vector.tensor_scalar(
                    d2b[:], xcs2[:, 0:NUM_SAM],
                    xcs2[:, NUM_SAM : NUM_SAM + 1], tvec2[:],
                    mybir.AluOpType.subtract, mybir.AluOpType.mult,
                )
                nc.vector.tensor_mul(lhsB[:], d2b[:], d2b[:])

            ab_tiles = []
            last_ab_mm = last_ab_exp = None
            if fact:
                # A/B tables: per sg, A sums at psum cols [sg*512, +32),
                # B sums at [sg*512+32, +288) (one bank per sg), then one
                # exp per sg -> ab_sb[sg] = [A (32 cols) | B (256 cols)] bf16
                ps_ab = ppool.tile([128, EXP_N], F32, tag="ps")
                for sg in range(N_SG):
                    s0 = sg * 512
                    nc.tensor.matmul(
                        ps_ab[:, s0 : s0 + HI_PC],
                        lhs_b[0 : K // 2, sg * 128 : (sg + 1) * 128],
                        oha[:],
                        start=True,
                        stop=True,
                    )
                    last_ab_mm = nc.tensor.matmul(
                        ps_ab[:, s0 + HI_PC : s0 + HI_PC + N_LO],
                        lhsB[:, sg * 128 : (sg + 1) * 128],
                        ohb[:],
                        start=True,
                        stop=True,
                    )
                for sg in range(N_SG):
                    ab = cpool.tile([128, HI_PC + N_LO], BF16, tag=f"ab{sg}")
                    s0 = sg * 512
                    if DEBUG_SKIP_AB_EXP:
                        nc.vector.memset(ab[:], 0.5)
                    else:
                        last_ab_exp = nc.scalar.activation(
                            ab[:], ps_ab[:, s0 : s0 + HI_PC + N_LO], Exp
                        )
                    ab_tiles.append(ab)

            # ---- main loop ----
            # fact: groups 0,2 via onehot-matmul + ACT exp; groups 1,3 via
            # DVE broadcast multiply A[s,hi]*B[s,lo] (no exp, no big matmul)
            dve_groups = {1, 3} if fact else set()
            for sg in range(N_SG):
                lhsT = lhs_b[0:K, sg * 128 : (sg + 1) * 128]  # [32, 128]
                for g in range(N_GRP):
                    stg = spool.tile([128, EXP_N], OUT_DT)
                    out_slice = out_ext[
                        sg * 128 : (sg + 1) * 128, g * EXP_N : (g + 1) * EXP_N
                    ]
                    if g in dve_groups:
                        ab = ab_tiles[sg]
                        Ab = (
                            ab[:, g * 8 : (g + 1) * 8]
                            .rearrange("p (h o) -> p h o", o=1)
                            .broadcast_to([128, 8, N_LO])
                        )
                        Bb = (
                            ab[:, HI_PC : HI_PC + N_LO]
                            .rearrange("p (o n) -> p o n", o=1)
                            .broadcast_to([128, 8, N_LO])
                        )
                        o3 = stg[:].rearrange("p (h n) -> p h n", h=8)
                        nc.vector.tensor_tensor(o3, Bb, Ab, mybir.AluOpType.mult)
                        nc.sync.dma_start(out=out_slice, in_=stg[:])
                        continue
                    ps = ppool.tile([128, EXP_N], F32, tag="ps")
                    for j in range(N_MM):
                        rt = g * N_MM + j
                        mm = nc.tensor.matmul(
                            ps[:, j * MM_N : (j + 1) * MM_N],
                            lhsT,
                            oh[:, rt * MM_N : (rt + 1) * MM_N],
                            start=True,
                            stop=True,
                        )
                        if last_ab_mm is not None:
                            # keep the AB phase ahead of the loop on the PE
                            # stream so the shared psum slot can't deadlock
                            add_dep_helper(
                                mm.ins, last_ab_mm.ins, sync=False,
                                reason="AB tables before rule matmuls",
                            )
                            last_ab_mm = None
                    if last_ab_exp is not None:
                        ex = nc.scalar.activation(
                            stg[:, 0:MM_N], ps[:, 0:MM_N], Exp
                        )
                        add_dep_helper(
                            ex.ins, last_ab_exp.ins, sync=False,
                            reason="AB exps before rule exps",
                        )
                        last_ab_exp = None
                        nc.scalar.activation(stg[:, MM_N:], ps[:, MM_N:], Exp)
                        nc.sync.dma_start(out=out_slice, in_=stg[:])
                    elif sg == N_SG - 1 and g == N_GRP - 1:
                        nc.scalar.activation(stg[:], ps[:], Exp)
                        # two half DMAs run concurrently -> shorter drain tail
                        h = EXP_N // 2
                        nc.sync.dma_start(out=out_slice[:, :h], in_=stg[:, :h])
                        nc.sync.dma_start(out=out_slice[:, h:], in_=stg[:, h:])
                    else:
                        nc.scalar.activation(stg[:], ps[:], Exp)
                        nc.sync.dma_start(out=out_slice, in_=stg[:])

    nc.compile()
    return nc


KX = K + K // 2       # xcs rows: 32 standard + 16 duplicated factor-B rows
D_A = IN_DIM // 2     # factor A: dims 0..3 (k rows 0..15)
N_HI = NUM_FS**D_A    # 256 A-codes; per core 32 hi blocks
N_LO = NUM_FS**D_A    # 256 B-codes
HI_PC = RPC // N_LO   # 32 hi blocks per core


def _is_factorizable(fs):
    """fs[r, 0:4] depends only on r>>8 and fs[r, 4:8] only on r&255
    (true for the FuCo-FRB cartesian rule base)."""
    a = fs[:, :D_A].reshape(N_HI, N_LO, D_A)
    b = fs[:, D_A:].reshape(N_HI, N_LO, D_A)
    return bool((a == a[:, :1]).all() and (b == b[:1]).all())


def _prep_in_maps(model_input, center, spread, fs_ind):
    model_input = np.ascontiguousarray(model_input, dtype=np.float32)
    center = np.ascontiguousarray(center, dtype=np.float32)
    spread = np.ascontiguousarray(spread, dtype=np.float32)
    fs = np.clip(np.asarray(fs_ind), 0, NUM_FS - 1).astype(np.int64)

    # one-hot with k = d*NUM_FS + f rows: oh[k, r] = -1 iff fs_ind[r, d] == f
    # (the -1 carries the gaussian exponent's sign)
    oh = np.zeros((K, NUM_RULE), dtype=ml_dtypes.bfloat16)
    r = np.arange(NUM_RULE)
    for d in range(IN_DIM):
        oh[d * NUM_FS + fs[:, d], r] = -1.0

    # xcs: x transposed/repeated over f, plus center and spread columns
    # (row k = d*4+f holds x[s, d], center[f, d], spread[f, d]).
    # Rows 32..47 duplicate rows 16..31 (factor-B dims) so the fast path's
    # K=16 B-matmul can run at base partition 32 (tile_position constraint).
    xcs = np.empty((K, NUM_SAM + 2), dtype=np.float32)
    xcs[:, :NUM_SAM] = np.repeat(model_input.T, NUM_FS, axis=0)
    xcs[:, NUM_SAM] = center.T.reshape(K)
    xcs[:, NUM_SAM + 1] = spread.T.reshape(K)

    fact = _is_factorizable(fs)
    oha = ohb = None
    if fact:
        # A-table one-hot [16, 256 hi codes], B-table one-hot [16, 256]
        oha = np.zeros((K // 2, N_HI), dtype=ml_dtypes.bfloat16)
        ohb = np.zeros((K // 2, N_LO), dtype=ml_dtypes.bfloat16)
        hi_rep = fs[:: N_LO, :D_A]  # [256, 4] representative rows
        lo_rep = fs[:N_LO, D_A:]    # [256, 4]
        for d in range(D_A):
            oha[d * NUM_FS + hi_rep[:, d], np.arange(N_HI)] = -1.0
            ohb[d * NUM_FS + lo_rep[:, d], np.arange(N_LO)] = -1.0

    maps = []
    for i in range(N_CORES):
        m = {
            "onehot": np.ascontiguousarray(oh[:, i * RPC : (i + 1) * RPC]),
            "xcs": xcs,
        }
        if fact:
            m["oha"] = np.ascontiguousarray(oha[:, i * HI_PC : (i + 1) * HI_PC])
            m["ohb"] = ohb
        maps.append(m)
    return fact, maps


def _run(inputs, trace=False, **spmd_kwargs):
    fact, in_maps = _prep_in_maps(
        inputs["model_input"], inputs["center"], inputs["spread"], inputs["fs_ind"]
    )
    nc = build_nc(fact)
    res = run_bass_kernel_spmd(
        nc, in_maps, core_ids=list(range(N_CORES)), trace=trace, **spmd_kwargs
    )
    out = np.concatenate(
        [res.results[i]["out"].astype(np.float32) for i in range(N_CORES)], axis=1
    )
    return out, res


def kernel(model_input, center, spread, fs_ind):
    out, _ = _run(
        {
            "model_input": model_input,
            "center": center,
            "spread": spread,
            "fs_ind": fs_ind,
        }
    )
    return out


# revision 63
# speedup vs baseline: 1.0217x; 1.0193x over previous
"""Trainium2 Bass kernel for the Antecedent (fuzzy firing strength) problem.

fir[s, r] = exp(sum_d logmv[s, fs_ind[r, d], d])
with logmv[s, f, d] = -(x[s,d] - c[f,d])^2 / (2 * spread[f,d]^2)

The gather+sum over d is a matmul with contraction K = num_fs*in_dim = 32:
    fir[s, r] = exp( sum_k oh[k, r] * d2sq[k, s] ),
    oh[f*8+d, r]   = -1 iff fs_ind[r, d] == f, else 0 (host-built bf16 index
                     encoding; the -1 carries the gaussian exponent's sign)
    d2sq[f*8+d, s] = ((x[s,d]-c[f,d]) / (spread[f,d]*sqrt(2)))^2
                     (device-computed from x/center/spread)

Sharding: rules split across the 8 cores (8192 rules each); samples replicated.
Per core: 64 bf16 matmuls [K=32, M=128 samples, N=512 rules] -> f32 PSUM,
ScalarE Exp PSUM[128,2048] -> bf16 SBUF, 0.5MB DMAs to the [512, 8192] output
slice (bf16, upcast to f32 on the host). Steady state is ScalarE-bound: exp
runs at 1 elem/cycle/lane, 4.2M output elems/core ~= 31us, with matmul (~90%)
and output DMA (~60%) hidden under it.
"""

import sys

if "/opt/trn_rl_repo" not in sys.path:
    sys.path.insert(0, "/opt/trn_rl_repo")

import ml_dtypes
import numpy as np

import concourse.bacc as bacc
import concourse.mybir as mybir
import concourse.tile as tile
from concourse.bass_utils import run_bass_kernel_spmd
from concourse.tile_rust import add_dep_helper

NUM_SAM = 512
IN_DIM = 8
NUM_FS = 4
NUM_RULE = 65536
K = NUM_FS * IN_DIM  # 32 contraction size
N_CORES = 8
RPC = NUM_RULE // N_CORES  # 8192 rules per core

F32 = mybir.dt.float32
BF16 = mybir.dt.bfloat16
OUT_DT = BF16  # fir values are exp(<=0) in (0,1]; bf16 keeps rel err ~1e-3

# loop tiling (per core)
N_SG = NUM_SAM // 128          # 4 sample groups of 128 (partition dim)
N_MM = 4                       # matmuls per exp group (512 rules)
MM_N = 512                     # moving free dim per matmul
EXP_N = N_MM * MM_N            # 2048 rules per exp + output DMA group
N_GRP = RPC // EXP_N           # 4 groups per sample group


def build_nc(fact):
    nc = bacc.Bacc("TRN2", target_bir_lowering=False, debug=False, num_devices=N_CORES)

    oh_ext = nc.dram_tensor("onehot", [K, RPC], BF16, kind="ExternalInput")
    # xcs: cols 0..NUM_SAM-1 = x[s,d] repeated over f; col NUM_SAM = center,
    # col NUM_SAM+1 = spread (single input DMA for the whole prologue)
    xcs_ext = nc.dram_tensor("xcs", [K, NUM_SAM + 2], F32, kind="ExternalInput")
    if fact:
        oha_ext = nc.dram_tensor("oha", [K // 2, HI_PC], BF16, kind="ExternalInput")
        ohb_ext = nc.dram_tensor("ohb", [K // 2, N_LO], BF16, kind="ExternalInput")
    out_ext = nc.dram_tensor("out", [NUM_SAM, RPC], OUT_DT, kind="ExternalOutput")

    with tile.TileContext(nc) as tc:
        with (
            tc.tile_pool(name="const", bufs=1) as cpool,
            tc.tile_pool(name="stage", bufs=4) as spool,
            tc.tile_pool(name="psum", bufs=2, space="PSUM") as ppool,
        ):
            # ---- prologue: tiny inputs + membership table ----
            xcs = cpool.tile([K, NUM_SAM + 2], F32)
            nc.sync.dma_start(out=xcs[:], in_=xcs_ext[:])
            xt32 = xcs[:, 0:NUM_SAM]
            cvec = xcs[:, NUM_SAM : NUM_SAM + 1]
            svec = xcs[:, NUM_SAM + 1 : NUM_SAM + 2]

            if fact:
                # factor one-hots, plus a base-partition-0 copy of the
                # factor-B rows of xcs (matmul operands need bp 0)
                oha = cpool.tile([K // 2, HI_PC], BF16)
                ohb = cpool.tile([K // 2, N_LO], BF16)
                xcs2 = cpool.tile([K // 2, NUM_SAM + 2], F32)
                nc.sync.dma_start(out=oha[:], in_=oha_ext[:])
                nc.sync.dma_start(out=ohb[:], in_=ohb_ext[:])
                nc.sync.dma_start(out=xcs2[:], in_=xcs_ext[K // 2 : K, :])

            # one-hot rule encoding [K, RPC] with entries -1 (carries the
            # minus sign of the gaussian exponent); issued on the Scalar
            # HWDGE queue so it doesn't serialize behind Sync's const DMA
            oh = cpool.tile([K, RPC], BF16)
            c0 = 0
            for csz in (MM_N, 2560, 2560, 2560):  # small first chunk: the
                nc.scalar.dma_start(             # first matmul starts sooner
                    out=oh[:, c0 : c0 + csz],
                    in_=oh_ext[:, c0 : c0 + csz],
                )
                c0 += csz

            # d2[k, s] = (x - c) / (s * sqrt(2)); lhsT = d2^2 (bf16).
            # The exponent's minus sign lives in the -1 one-hot entries.
            rsv = cpool.tile([K, 1], F32)
            tvec = cpool.tile([K, 1], F32)
            nc.vector.reciprocal(rsv[:], svec)
            nc.vector.tensor_scalar_mul(tvec[:], rsv[:], 0.7071067811865476)
            d2 = cpool.tile([K, NUM_SAM], F32)
            lhs_b = cpool.tile([K, NUM_SAM], BF16)
            # sample-group 0 first so its matmuls can start early
            for sl in (slice(0, 128), slice(128, NUM_SAM)):
                nc.vector.tensor_scalar(
                    d2[:, sl], xt32[:, sl], cvec, tvec[:],
                    mybir.AluOpType.subtract, mybir.AluOpType.mult,
                )
                nc.vector.tensor_mul(lhs_b[:, sl], d2[:, sl], d2[:, sl])

            Exp = mybir.ActivationFunctionType.Exp

            lhsB = None
            if fact:
                rsv2 = cpool.tile([K // 2, 1], F32)
                tvec2 = cpool.tile([K // 2, 1], F32)
                nc.vector.reciprocal(rsv2[:], xcs2[:, NUM_SAM + 1 : NUM_SAM + 2])
                nc.vector.tensor_scalar_mul(tvec2[:], rsv2[:], 0.7071067811865476)
                d2b = cpool.tile([K // 2, NUM_SAM], F32)
                lhsB = cpool.tile([K // 2, NUM_SAM], BF16)
                nc.vector.tensor_scalar(
                    d2b[:], xcs2[:, 0:NUM_SAM],
                    xcs2[:, NUM_SAM : NUM_SAM + 1], tvec2[:],
                    mybir.AluOpType.subtract, mybir.AluOpType.mult,
                )
                nc.vector.tensor_mul(lhsB[:], d2b[:], d2b[:])

            ab_tiles = []
            last_ab_mm = last_ab_exp = None
            if fact:
                # A/B tables: per sg, A sums at psum cols [sg*512, +32),
                # B sums at [sg*512+32, +288) (one bank per sg), then one
                # exp per sg -> ab_sb[sg] = [A (32 cols) | B (256 cols)] bf16
                ps_ab = ppool.tile([128, EXP_N], F32, tag="ps")
                for sg in range(N_SG):
                    s0 = sg * 512
                    nc.tensor.matmul(
                        ps_ab[:, s0 : s0 + HI_PC],
                        lhs_b[0 : K // 2, sg * 128 : (sg + 1) * 128],
                        oha[:],
                        start=True,
                        stop=True,
                    )
                    last_ab_mm = nc.tensor.matmul(
                        ps_ab[:, s0 + HI_PC : s0 + HI_PC + N_LO],
                        lhsB[:, sg * 128 : (sg + 1) * 128],
                        ohb[:],
                        start=True,
                        stop=True,
                    )
                for sg in range(N_SG):
                    ab = cpool.tile([128, HI_PC + N_LO], BF16, tag=f"ab{sg}")
                    s0 = sg * 512
                    last_ab_exp = nc.scalar.activation(
                        ab[:], ps_ab[:, s0 : s0 + HI_PC + N_LO], Exp
                    )
                    ab_tiles.append(ab)

            # ---- main loop ----
            # fact: groups 0,2 via onehot-matmul + ACT exp; groups 1,3 via
            # DVE broadcast multiply A[s,hi]*B[s,lo] (no exp, no big matmul)
            dve_groups = {1, 3} if fact else set()
            for sg in range(N_SG):
                lhsT = lhs_b[0:K, sg * 128 : (sg + 1) * 128]  # [32, 128]
                for g in range(N_GRP):
                    stg = spool.tile([128, EXP_N], OUT_DT)
                    out_slice = out_ext[
                        sg * 128 : (sg + 1) * 128, g * EXP_N : (g + 1) * EXP_N
                    ]
                    if g in dve_groups:
                        ab = ab_tiles[sg]
                        Ab = (
                            ab[:, g * 8 : (g + 1) * 8]
                            .rearrange("p (h o) -> p h o", o=1)
                            .broadcast_to([128, 8, N_LO])
                        )
                        Bb = (
                            ab[:, HI_PC : HI_PC + N_LO]
                            .rearrange("p (o n) -> p o n", o=1)
                            .broadcast_to([128, 8, N_LO])
                        )
                        o3 = stg[:].rearrange("p (h n) -> p h n", h=8)
                        nc.vector.tensor_tensor(o3, Bb, Ab, mybir.AluOpType.mult)
                        nc.sync.dma_start(out=out_slice, in_=stg[:])
                        continue
                    ps = ppool.tile([128, EXP_N], F32, tag="ps")
                    for j in range(N_MM):
                        rt = g * N_MM + j
                        mm = nc.tensor.matmul(
                            ps[:, j * MM_N : (j + 1) * MM_N],
                            lhsT,
                            oh[:, rt * MM_N : (rt + 1) * MM_N],
                            start=True,
                            stop=True,
                        )
                        if last_ab_mm is not None:
                            # keep the AB phase ahead of the loop on the PE
                            # stream so the shared psum slot can't deadlock
                            add_dep_helper(
                                mm.ins, last_ab_mm.ins, sync=False,
                                reason="AB tables before rule matmuls",
                            )
                            last_ab_mm = None
                    if last_ab_exp is not None:
                        ex = nc.scalar.activation(
                            stg[:, 0:MM_N], ps[:, 0:MM_N], Exp
                        )
                        add_dep_helper(
                            ex.ins, last_ab_exp.ins, sync=False,
                            reason="AB exps before rule exps",
                        )
                        last_ab_exp = None
                        nc.scalar.activation(stg[:, MM_N:], ps[:, MM_N:], Exp)
                        nc.sync.dma_start(out=out_slice, in_=stg[:])
                    elif sg == N_SG - 1 and g == N_GRP - 1:
                        nc.scalar.activation(stg[:], ps[:], Exp)
                        # two half DMAs run concurrently -> shorter drain tail
                        h = EXP_N // 2
                        nc.sync.dma_start(out=out_slice[:, :h], in_=stg[:, :h])
                        nc.sync.dma_start(out=out_slice[:, h:], in_=stg[:, h:])
                    else:
                        nc.scalar.activation(stg[:], ps[:], Exp)
                        nc.sync.dma_start(out=out_slice, in_=stg[:])

    nc.compile()
    return nc


KX = K + K // 2       # xcs rows: 32 standard + 16 duplicated factor-B rows
D_A = IN_DIM // 2     # factor A: dims 0..3 (k rows 0..15)
N_HI = NUM_FS**D_A    # 256 A-codes; per core 32 hi blocks
N_LO = NUM_FS**D_A    # 256 B-codes
HI_PC = RPC // N_LO   # 32 hi blocks per core


def _is_factorizable(fs):
    """fs[r, 0:4] depends only on r>>8 and fs[r, 4:8] only on r&255
    (true for the FuCo-FRB cartesian rule base)."""
    a = fs[:, :D_A].reshape(N_HI, N_LO, D_A)
    b = fs[:, D_A:].reshape(N_HI, N_LO, D_A)
    return bool((a == a[:, :1]).all() and (b == b[:1]).all())


def _prep_in_maps(model_input, center, spread, fs_ind):
    model_input = np.ascontiguousarray(model_input, dtype=np.float32)
    center = np.ascontiguousarray(center, dtype=np.float32)
    spread = np.ascontiguousarray(spread, dtype=np.float32)
    fs = np.clip(np.asarray(fs_ind), 0, NUM_FS - 1).astype(np.int64)

    # one-hot with k = d*NUM_FS + f rows: oh[k, r] = -1 iff fs_ind[r, d] == f
    # (the -1 carries the gaussian exponent's sign)
    oh = np.zeros((K, NUM_RULE), dtype=ml_dtypes.bfloat16)
    r = np.arange(NUM_RULE)
    for d in range(IN_DIM):
        oh[d * NUM_FS + fs[:, d], r] = -1.0

    # xcs: x transposed/repeated over f, plus center and spread columns
    # (row k = d*4+f holds x[s, d], center[f, d], spread[f, d]).
    # Rows 32..47 duplicate rows 16..31 (factor-B dims) so the fast path's
    # K=16 B-matmul can run at base partition 32 (tile_position constraint).
    xcs = np.empty((K, NUM_SAM + 2), dtype=np.float32)
    xcs[:, :NUM_SAM] = np.repeat(model_input.T, NUM_FS, axis=0)
    xcs[:, NUM_SAM] = center.T.reshape(K)
    xcs[:, NUM_SAM + 1] = spread.T.reshape(K)

    fact = _is_factorizable(fs)
    oha = ohb = None
    if fact:
        # A-table one-hot [16, 256 hi codes], B-table one-hot [16, 256]
        oha = np.zeros((K // 2, N_HI), dtype=ml_dtypes.bfloat16)
        ohb = np.zeros((K // 2, N_LO), dtype=ml_dtypes.bfloat16)
        hi_rep = fs[:: N_LO, :D_A]  # [256, 4] representative rows
        lo_rep = fs[:N_LO, D_A:]    # [256, 4]
        for d in range(D_A):
            oha[d * NUM_FS + hi_rep[:, d], np.arange(N_HI)] = -1.0
            ohb[d * NUM_FS + lo_rep[:, d], np.arange(N_LO)] = -1.0

    maps = []
    for i in range(N_CORES):
        m = {
            "onehot": np.ascontiguousarray(oh[:, i * RPC : (i + 1) * RPC]),
            "xcs": xcs,
        }
        if fact:
            m["oha"] = np.ascontiguousarray(oha[:, i * HI_PC : (i + 1) * HI_PC])
            m["ohb"] = ohb
        maps.append(m)
    return fact, maps


def _run(inputs, trace=False, **spmd_kwargs):
    fact, in_maps = _prep_in_maps(
        inputs["model_input"], inputs["center"], inputs["spread"], inputs["fs_ind"]
    )
    nc = build_nc(fact)
    res = run_bass_kernel_spmd(
        nc, in_maps, core_ids=list(range(N_CORES)), trace=trace, **spmd_kwargs
    )
    out = np.concatenate(
        [res.results[i]["out"].astype(np.float32) for i in range(N_CORES)], axis=1
    )
    return out, res


def kernel(model_input, center, spread, fs_ind):
    out, _ = _run(
        {
            "model_input": model_input,
            "center": center,
            "spread": spread,
            "fs_ind": fs_ind,
        }
    )
    return out


# revision 64
# speedup vs baseline: 1.0249x; 1.0032x over previous
"""Trainium2 Bass kernel for the Antecedent (fuzzy firing strength) problem.

fir[s, r] = exp(sum_d logmv[s, fs_ind[r, d], d])
with logmv[s, f, d] = -(x[s,d] - c[f,d])^2 / (2 * spread[f,d]^2)

The gather+sum over d is a matmul with contraction K = num_fs*in_dim = 32:
    fir[s, r] = exp( sum_k oh[k, r] * d2sq[k, s] ),
    oh[f*8+d, r]   = -1 iff fs_ind[r, d] == f, else 0 (host-built bf16 index
                     encoding; the -1 carries the gaussian exponent's sign)
    d2sq[f*8+d, s] = ((x[s,d]-c[f,d]) / (spread[f,d]*sqrt(2)))^2
                     (device-computed from x/center/spread)

Sharding: rules split across the 8 cores (8192 rules each); samples replicated.
Per core: 64 bf16 matmuls [K=32, M=128 samples, N=512 rules] -> f32 PSUM,
ScalarE Exp PSUM[128,2048] -> bf16 SBUF, 0.5MB DMAs to the [512, 8192] output
slice (bf16, upcast to f32 on the host). Steady state is ScalarE-bound: exp
runs at 1 elem/cycle/lane, 4.2M output elems/core ~= 31us, with matmul (~90%)
and output DMA (~60%) hidden under it.
"""

import sys

if "/opt/trn_rl_repo" not in sys.path:
    sys.path.insert(0, "/opt/trn_rl_repo")

import ml_dtypes
import numpy as np

import concourse.bacc as bacc
import concourse.mybir as mybir
import concourse.tile as tile
from concourse.bass_utils import run_bass_kernel_spmd
from concourse.tile_rust import add_dep_helper

NUM_SAM = 512
IN_DIM = 8
NUM_FS = 4
NUM_RULE = 65536
K = NUM_FS * IN_DIM  # 32 contraction size
N_CORES = 8
RPC = NUM_RULE // N_CORES  # 8192 rules per core

F32 = mybir.dt.float32
BF16 = mybir.dt.bfloat16
OUT_DT = BF16  # fir values are exp(<=0) in (0,1]; bf16 keeps rel err ~1e-3

# loop tiling (per core)
N_SG = NUM_SAM // 128          # 4 sample groups of 128 (partition dim)
N_MM = 4                       # matmuls per exp group (512 rules)
MM_N = 512                     # moving free dim per matmul
EXP_N = N_MM * MM_N            # 2048 rules per exp + output DMA group
N_GRP = RPC // EXP_N           # 4 groups per sample group


def build_nc(fact):
    nc = bacc.Bacc("TRN2", target_bir_lowering=False, debug=False, num_devices=N_CORES)

    oh_ext = nc.dram_tensor("onehot", [K, RPC], BF16, kind="ExternalInput")
    # xcs: cols 0..NUM_SAM-1 = x[s,d] repeated over f; col NUM_SAM = center,
    # col NUM_SAM+1 = spread (single input DMA for the whole prologue)
    xcs_ext = nc.dram_tensor("xcs", [K, NUM_SAM + 2], F32, kind="ExternalInput")
    if fact:
        oha_ext = nc.dram_tensor("oha", [K // 2, HI_PC], BF16, kind="ExternalInput")
        ohb_ext = nc.dram_tensor("ohb", [K // 2, N_LO], BF16, kind="ExternalInput")
    out_ext = nc.dram_tensor("out", [NUM_SAM, RPC], OUT_DT, kind="ExternalOutput")

    with tile.TileContext(nc) as tc:
        with (
            tc.tile_pool(name="const", bufs=1) as cpool,
            tc.tile_pool(name="stage", bufs=4) as spool,
            tc.tile_pool(name="psum", bufs=2, space="PSUM") as ppool,
        ):
            # ---- prologue: tiny inputs + membership table ----
            xcs = cpool.tile([K, NUM_SAM + 2], F32)
            nc.sync.dma_start(out=xcs[:], in_=xcs_ext[:])
            xt32 = xcs[:, 0:NUM_SAM]
            cvec = xcs[:, NUM_SAM : NUM_SAM + 1]
            svec = xcs[:, NUM_SAM + 1 : NUM_SAM + 2]

            if fact:
                # factor one-hots, plus a base-partition-0 copy of the
                # factor-B rows of xcs (matmul operands need bp 0)
                oha = cpool.tile([K // 2, HI_PC], BF16)
                ohb = cpool.tile([K // 2, N_LO], BF16)
                xcs2 = cpool.tile([K // 2, NUM_SAM + 2], F32)
                nc.sync.dma_start(out=oha[:], in_=oha_ext[:])
                nc.sync.dma_start(out=ohb[:], in_=ohb_ext[:])
                nc.sync.dma_start(out=xcs2[:], in_=xcs_ext[K // 2 : K, :])

            # one-hot rule encoding [K, RPC] with entries -1 (carries the
            # minus sign of the gaussian exponent); issued on the Scalar
            # HWDGE queue so it doesn't serialize behind Sync's const DMA
            oh = cpool.tile([K, RPC], BF16)
            if fact:
                # only the ACT-path groups (g=0,2 -> cols 0:2048, 4096:6144)
                # read oh; skip the DVE-path halves entirely
                chunks = [(0, MM_N), (MM_N, EXP_N - MM_N), (2 * EXP_N, EXP_N)]
            else:
                chunks = [(0, MM_N), (MM_N, 2560), (2688, 2560), (5248, 2944)]
            for c0, csz in chunks:  # small first chunk: first matmul sooner
                nc.scalar.dma_start(
                    out=oh[:, c0 : c0 + csz],
                    in_=oh_ext[:, c0 : c0 + csz],
                )

            # d2[k, s] = (x - c) / (s * sqrt(2)); lhsT = d2^2 (bf16).
            # The exponent's minus sign lives in the -1 one-hot entries.
            rsv = cpool.tile([K, 1], F32)
            tvec = cpool.tile([K, 1], F32)
            nc.vector.reciprocal(rsv[:], svec)
            nc.vector.tensor_scalar_mul(tvec[:], rsv[:], 0.7071067811865476)
            d2 = cpool.tile([K, NUM_SAM], F32)
            lhs_b = cpool.tile([K, NUM_SAM], BF16)
            # sample-group 0 first so its matmuls can start early
            for sl in (slice(0, 128), slice(128, NUM_SAM)):
                nc.vector.tensor_scalar(
                    d2[:, sl], xt32[:, sl], cvec, tvec[:],
                    mybir.AluOpType.subtract, mybir.AluOpType.mult,
                )
                nc.vector.tensor_mul(lhs_b[:, sl], d2[:, sl], d2[:, sl])

            Exp = mybir.ActivationFunctionType.Exp

            lhsB = None
            if fact:
                rsv2 = cpool.tile([K // 2, 1], F32)
                tvec2 = cpool.tile([K // 2, 1], F32)
                nc.vector.reciprocal(rsv2[:], xcs2[:, NUM_SAM + 1 : NUM_SAM + 2])
                nc.vector.tensor_scalar_mul(tvec2[:], rsv2[:], 0.7071067811865476)
                d2b = cpool.tile([K // 2, NUM_SAM], F32)
                lhsB = cpool.tile([K // 2, NUM_SAM], BF16)
                nc.vector.tensor_scalar(
                    d2b[:], xcs2[:, 0:NUM_SAM],
                    xcs2[:, NUM_SAM : NUM_SAM + 1], tvec2[:],
                    mybir.AluOpType.subtract, mybir.AluOpType.mult,
                )
                nc.vector.tensor_mul(lhsB[:], d2b[:], d2b[:])

            ab_tiles = []
            last_ab_mm = last_ab_exp = None
            if fact:
                # A/B tables: per sg, A sums at psum cols [sg*512, +32),
                # B sums at [sg*512+32, +288) (one bank per sg), then one
                # exp per sg -> ab_sb[sg] = [A (32 cols) | B (256 cols)] bf16
                ps_ab = ppool.tile([128, EXP_N], F32, tag="ps")
                for sg in range(N_SG):
                    s0 = sg * 512
                    nc.tensor.matmul(
                        ps_ab[:, s0 : s0 + HI_PC],
                        lhs_b[0 : K // 2, sg * 128 : (sg + 1) * 128],
                        oha[:],
                        start=True,
                        stop=True,
                    )
                    last_ab_mm = nc.tensor.matmul(
                        ps_ab[:, s0 + HI_PC : s0 + HI_PC + N_LO],
                        lhsB[:, sg * 128 : (sg + 1) * 128],
                        ohb[:],
                        start=True,
                        stop=True,
                    )
                for sg in range(N_SG):
                    ab = cpool.tile([128, HI_PC + N_LO], BF16, tag=f"ab{sg}")
                    s0 = sg * 512
                    last_ab_exp = nc.scalar.activation(
                        ab[:], ps_ab[:, s0 : s0 + HI_PC + N_LO], Exp
                    )
                    ab_tiles.append(ab)

            # ---- main loop ----
            # fact: groups 0,2 via onehot-matmul + ACT exp; groups 1,3 via
            # DVE broadcast multiply A[s,hi]*B[s,lo] (no exp, no big matmul)
            dve_groups = {1, 3} if fact else set()
            for sg in range(N_SG):
                lhsT = lhs_b[0:K, sg * 128 : (sg + 1) * 128]  # [32, 128]
                for g in range(N_GRP):
                    stg = spool.tile([128, EXP_N], OUT_DT)
                    out_slice = out_ext[
                        sg * 128 : (sg + 1) * 128, g * EXP_N : (g + 1) * EXP_N
                    ]
                    if g in dve_groups:
                        ab = ab_tiles[sg]
                        Ab = (
                            ab[:, g * 8 : (g + 1) * 8]
                            .rearrange("p (h o) -> p h o", o=1)
                            .broadcast_to([128, 8, N_LO])
                        )
                        Bb = (
                            ab[:, HI_PC : HI_PC + N_LO]
                            .rearrange("p (o n) -> p o n", o=1)
                            .broadcast_to([128, 8, N_LO])
                        )
                        o3 = stg[:].rearrange("p (h n) -> p h n", h=8)
                        nc.vector.tensor_tensor(o3, Bb, Ab, mybir.AluOpType.mult)
                        nc.sync.dma_start(out=out_slice, in_=stg[:])
                        continue
                    ps = ppool.tile([128, EXP_N], F32, tag="ps")
                    for j in range(N_MM):
                        rt = g * N_MM + j
                        mm = nc.tensor.matmul(
                            ps[:, j * MM_N : (j + 1) * MM_N],
                            lhsT,
                            oh[:, rt * MM_N : (rt + 1) * MM_N],
                            start=True,
                            stop=True,
                        )
                        if last_ab_mm is not None:
                            # keep the AB phase ahead of the loop on the PE
                            # stream so the shared psum slot can't deadlock
                            add_dep_helper(
                                mm.ins, last_ab_mm.ins, sync=False,
                                reason="AB tables before rule matmuls",
                            )
                            last_ab_mm = None
                    if last_ab_exp is not None:
                        ex = nc.scalar.activation(
                            stg[:, 0:MM_N], ps[:, 0:MM_N], Exp
                        )
                        add_dep_helper(
                            ex.ins, last_ab_exp.ins, sync=False,
                            reason="AB exps before rule exps",
                        )
                        last_ab_exp = None
                        nc.scalar.activation(stg[:, MM_N:], ps[:, MM_N:], Exp)
                        nc.sync.dma_start(out=out_slice, in_=stg[:])
                    elif sg == N_SG - 1 and g == N_GRP - 1:
                        nc.scalar.activation(stg[:], ps[:], Exp)
                        # two half DMAs run concurrently -> shorter drain tail
                        h = EXP_N // 2
                        nc.sync.dma_start(out=out_slice[:, :h], in_=stg[:, :h])
                        nc.sync.dma_start(out=out_slice[:, h:], in_=stg[:, h:])
                    else:
                        nc.scalar.activation(stg[:], ps[:], Exp)
                        nc.sync.dma_start(out=out_slice, in_=stg[:])

    nc.compile()
    return nc


KX = K + K // 2       # xcs rows: 32 standard + 16 duplicated factor-B rows
D_A = IN_DIM // 2     # factor A: dims 0..3 (k rows 0..15)
N_HI = NUM_FS**D_A    # 256 A-codes; per core 32 hi blocks
N_LO = NUM_FS**D_A    # 256 B-codes
HI_PC = RPC // N_LO   # 32 hi blocks per core


def _is_factorizable(fs):
    """fs[r, 0:4] depends only on r>>8 and fs[r, 4:8] only on r&255
    (true for the FuCo-FRB cartesian rule base)."""
    a = fs[:, :D_A].reshape(N_HI, N_LO, D_A)
    b = fs[:, D_A:].reshape(N_HI, N_LO, D_A)
    return bool((a == a[:, :1]).all() and (b == b[:1]).all())


def _prep_in_maps(model_input, center, spread, fs_ind):
    model_input = np.ascontiguousarray(model_input, dtype=np.float32)
    center = np.ascontiguousarray(center, dtype=np.float32)
    spread = np.ascontiguousarray(spread, dtype=np.float32)
    fs = np.clip(np.asarray(fs_ind), 0, NUM_FS - 1).astype(np.int64)

    # one-hot with k = d*NUM_FS + f rows: oh[k, r] = -1 iff fs_ind[r, d] == f
    # (the -1 carries the gaussian exponent's sign)
    oh = np.zeros((K, NUM_RULE), dtype=ml_dtypes.bfloat16)
    r = np.arange(NUM_RULE)
    for d in range(IN_DIM):
        oh[d * NUM_FS + fs[:, d], r] = -1.0

    # xcs: x transposed/repeated over f, plus center and spread columns
    # (row k = d*4+f holds x[s, d], center[f, d], spread[f, d]).
    # Rows 32..47 duplicate rows 16..31 (factor-B dims) so the fast path's
    # K=16 B-matmul can run at base partition 32 (tile_position constraint).
    xcs = np.empty((K, NUM_SAM + 2), dtype=np.float32)
    xcs[:, :NUM_SAM] = np.repeat(model_input.T, NUM_FS, axis=0)
    xcs[:, NUM_SAM] = center.T.reshape(K)
    xcs[:, NUM_SAM + 1] = spread.T.reshape(K)

    fact = _is_factorizable(fs)
    oha = ohb = None
    if fact:
        # A-table one-hot [16, 256 hi codes], B-table one-hot [16, 256]
        oha = np.zeros((K // 2, N_HI), dtype=ml_dtypes.bfloat16)
        ohb = np.zeros((K // 2, N_LO), dtype=ml_dtypes.bfloat16)
        hi_rep = fs[:: N_LO, :D_A]  # [256, 4] representative rows
        lo_rep = fs[:N_LO, D_A:]    # [256, 4]
        for d in range(D_A):
            oha[d * NUM_FS + hi_rep[:, d], np.arange(N_HI)] = -1.0
            ohb[d * NUM_FS + lo_rep[:, d], np.arange(N_LO)] = -1.0

    maps = []
    for i in range(N_CORES):
        m = {
            "onehot": np.ascontiguousarray(oh[:, i * RPC : (i + 1) * RPC]),
            "xcs": xcs,
        }
        if fact:
            m["oha"] = np.ascontiguousarray(oha[:, i * HI_PC : (i + 1) * HI_PC])
            m["ohb"] = ohb
        maps.append(m)
    return fact, maps


def _run(inputs, trace=False, **spmd_kwargs):
    fact, in_maps = _prep_in_maps(
        inputs["model_input"], inputs["center"], inputs["spread"], inputs["fs_ind"]
    )
    nc = build_nc(fact)
    res = run_bass_kernel_spmd(
        nc, in_maps, core_ids=list(range(N_CORES)), trace=trace, **spmd_kwargs
    )
    out = np.concatenate(
        [res.results[i]["out"].astype(np.float32) for i in range(N_CORES)], axis=1
    )
    return out, res


def kernel(model_input, center, spread, fs_ind):
    out, _ = _run(
        {
            "model_input": model_input,
            "center": center,
            "spread": spread,
            "fs_ind": fs_ind,
        }
    )
    return out
